# revision 24
# baseline (speedup 1.0000x reference)
"""Trainium2 Bass kernel for a graph-GRU (GRNN) forecast model.

Math (per batch b, node m, hidden h; N=2048, H=64, F=2, T=12, P=6):
  ht[b,m,:] = sum_n adj[n,m] * h[b,:,n]           (graph diffusion + transpose)
  r = sig(ht@Ur^T + inp@Wr^T + br); z = sig(...); nw = tanh(r*(ht@Un^T+bn1) + inp@Wn^T + bn2)
  h' = (1-z)*nw + z*ht
Encoder: inp = x_t (T steps). Decoder: out = fc(h); inp = [out, 0] (P steps).

Data-parallel over batch: 8 cores x 8 batches. Per-core layouts:
  hT8 [n=2048(part, 8 DR pair-tiles), (b,h)=512(free)] fp8 = h/3.75
  hS  [(bl,h)=128(part), pt=4, m=2048]  bf16
with b = 2*pt + bl.

Diffusion: adj = (I + 0.3*mask)/colsum. The fp8 mask holds EXACT values
offd=1.125, diag+=3.75 and the state is scaled h/3.75, so
  dps = (h/3.75) @ maskI = (h + 0.3*mask@h)  exactly in coefficient
  ht  = dps * dinv[m]     one DVE mult (no separate +h add)
Per (pt, m-1024-group): 8 DR matmuls, one LDWEIGHTS per kt2 (the
duplicate LDW for the second 512-chunk is removed by post-schedule
surgery in _dedup_ldweights).

Gates: rz 2-bank PSUM pair -> ONE sigmoid ACT [128,1024] with strided
dst (r/z planes of rz_row). All gate biases are folded into the input
matmuls via a ones-row (x/out staged at partitions 32g..32g+16 for
3-row-group-concurrent input matmuls). nw tanh runs [128,1024] per
group. Combine d/e/hn run [128,1024] split DVE/GpSimd. Transposes
(PE+identity) lag 2 units; scalar copies tp->fp8 with scale 1/3.75.
"""

import numpy as np
import ml_dtypes

B, T, F, N, H, P = 64, 12, 2, 2048, 64, 6
NCORES = 8
BC = B // NCORES          # batches per core = 8
NPT = BC // 2             # batch-pair tiles = 4
KT2 = 8                   # DoubleRow pair tiles
NMC = 4                   # m chunks of 512
NU = 8                    # units per step: (pt, grp) with grp = m-1024 half

_BF16 = ml_dtypes.bfloat16
_F8 = ml_dtypes.float8_e4m3fn
S8 = 1.0 / 3.75           # fp8 state scale

_compiled = None


def _ldw_sig(inst):
    try:
        ap = inst.ins[0]
        return (str(ap.memref), int(ap.offset), str(ap.ap), str(ap.dtype),
                str(inst.perf_mode), str(inst.is_transpose))
    except Exception:
        return None


def _dedup_ldweights(nc):
    """Remove InstLdweights that reload the identical stationary operand.

    Scans each block's (program-ordered) instruction list; an LDW whose
    lowered weights-AP matches the previous PE weight load — with only
    MATMUL/EventSemaphore PE instructions in between — is redundant.
    Only removes wait-free, update-free instances (keeps all sync
    semantics intact). Returns (#removed, #candidates).
    """
    removed = cand = 0
    for fn in nc.m.functions:
        for blk in fn.blocks:
            il = blk.instructions
            keep = []
            changed = False
            prev_sig = None
            for inst in il:
                t = type(inst).__name__
                eng = str(getattr(inst, 'engine', ''))
                if 'PE' not in eng:
                    keep.append(inst)
                    continue
                if t == 'InstLdweights':
                    sig = _ldw_sig(inst)
                    if sig is not None and sig == prev_sig:
                        cand += 1
                        try:
                            clean = (not inst.has_wait()
                                     and not inst.has_update())
                        except Exception:
                            clean = False
                        if clean:
                            removed += 1
                            changed = True
                            continue
                    prev_sig = sig
                elif t in ('InstMatmult', 'InstEventSemaphore'):
                    pass
                else:
                    prev_sig = None
                keep.append(inst)
            if changed:
                il.clear()
                il.extend(keep)
    return removed, cand


def _build_bass():
    import concourse.bass as bass
    import concourse.mybir as mybir
    from concourse import bacc
    import concourse.tile as tile

    bf16 = mybir.dt.bfloat16
    f32 = mybir.dt.float32
    f8 = mybir.dt.float8e4
    AF = mybir.ActivationFunctionType
    ALU = mybir.AluOpType
    DR = mybir.MatmulPerfMode.DoubleRow

    nc = bacc.Bacc(None, target_bir_lowering=False)

    x_d = nc.dram_tensor("xfull", [3, 17, T, N], bf16, kind="ExternalInput")
    h0_d = nc.dram_tensor("h0", [128, NPT, N], bf16, kind="ExternalInput")
    h0t_d = nc.dram_tensor("h0t8", [128, KT2, 2, 512], f8, kind="ExternalInput")
    mask_d = nc.dram_tensor("maskdr", [128, KT2, 2, N], f8, kind="ExternalInput")
    dinv_d = nc.dram_tensor("dinv", [128, N], bf16, kind="ExternalInput")
    ublk_d = nc.dram_tensor("ublk", [128, 3, 128], bf16, kind="ExternalInput")
    wx_d = nc.dram_tensor("wx17", [128, NPT, 128], bf16, kind="ExternalInput")
    r9_d = nc.dram_tensor("r9", [128, NPT, 128], bf16, kind="ExternalInput")
    fc_d = nc.dram_tensor("fcblk", [128, NPT, BC], bf16, kind="ExternalInput")
    bias_d = nc.dram_tensor("biases", [128, 2], f32, kind="ExternalInput")
    id_d = nc.dram_tensor("ident", [128, 128], bf16, kind="ExternalInput")
    out_d = nc.dram_tensor("out", [BC, P, N], bf16, kind="ExternalOutput")

    with tile.TileContext(nc) as tc:
        with (
            tc.tile_pool(name="const", bufs=1) as cp,
            tc.tile_pool(name="state", bufs=1) as sp,
            tc.tile_pool(name="rowp", bufs=2) as rp,
            tc.tile_pool(name="work", bufs=3) as wp,
            tc.tile_pool(name="xp", bufs=2) as xp,
            tc.tile_pool(name="dps", bufs=2, space="PSUM") as dpool,
            tc.tile_pool(name="rzps", bufs=1, space="PSUM") as rzpool,
            tc.tile_pool(name="nups", bufs=1, space="PSUM") as npool,
            tc.tile_pool(name="nwps", bufs=1, space="PSUM") as nwpool,
            tc.tile_pool(name="tps", bufs=1, space="PSUM") as tpool,
            tc.tile_pool(name="fcps", bufs=1, space="PSUM") as fcpool,
        ):
            h0s = sp.tile([128, NPT, N], bf16, tag="hS0", name="hS0")
            nc.sync.dma_start(h0s[:], h0_d[:])
            hT80 = sp.tile([128, KT2, 2, 512], f8, tag="hT0", name="hT0")
            nc.scalar.dma_start(hT80[:], h0t_d[:])
            ident = cp.tile([128, 128], bf16)
            nc.scalar.dma_start(ident[:], id_d[:])
            mask_sb = cp.tile([128, KT2, 2, N], f8)
            for kt2 in range(KT2):
                eng = nc.sync if kt2 % 2 else nc.scalar
                eng.dma_start(mask_sb[:, kt2, :, :], mask_d[:, kt2, :, :])
            dinv = cp.tile([128, N], bf16)
            nc.scalar.dma_start(dinv[:], dinv_d[:])
            ublk = cp.tile([128, 3, 128], bf16)
            nc.scalar.dma_start(ublk[:], ublk_d[:])
            wx17 = cp.tile([128, NPT, 128], bf16)
            nc.scalar.dma_start(wx17[:], wx_d[:])
            r9 = cp.tile([128, NPT, 128], bf16)
            nc.scalar.dma_start(r9[:], r9_d[:])
            fcblk = cp.tile([128, NPT, BC], bf16)
            nc.scalar.dma_start(fcblk[:], fc_d[:])
            biases = cp.tile([128, 2], f32)
            nc.scalar.dma_start(biases[:], bias_d[:])

            hS = [h0s, sp.tile([128, NPT, N], bf16, tag="hS1", name="hS1")]
            hT8 = [hT80, sp.tile([128, KT2, 2, 512], f8, tag="hT1", name="hT1")]
            # decoder staged outputs: partitions {32g..32g+7} data, {32g+8} ones
            osts = [sp.tile([128, N], bf16, tag=f"ost{i}", name=f"ost{i}")
                    for i in range(2)]
            for o in osts:
                for g in range(3):
                    # ones row lives at 32g+8; rows 32g..32g+7 are data
                    # (rewritten by the fc ACT each decoder step) and rows
                    # 9..31 of each group are never read.
                    nc.vector.memset(o[32 * g:32 * (g + 1), :], 1.0)

            pend_tr = []    # (src_state_idx s%2 of hSn, pt, g)
            pend_copy = []  # (tpl_tile, sidx, pt, g)

            def emit_tp_mms(sidx, pt, g):
                hSn_ = hS[sidx]
                tpl = tpool.tile([128, 4, 2, 128], bf16, tag="tp")
                for j in range(8):
                    nc.tensor.transpose(
                        tpl[:, j // 2, j % 2, :],
                        hSn_[:, pt, g * 1024 + j * 128: g * 1024 + (j + 1) * 128],
                        ident[:],
                    )
                pend_copy.append((tpl, sidx, pt, g))

            def emit_tp_copy():
                tpl, sidx, pt, g = pend_copy.pop(0)
                psl = slice(pt * 128, (pt + 1) * 128)
                nc.scalar.activation(
                    hT8[sidx][:, 4 * g:4 * g + 4, :, psl], tpl[:], AF.Copy,
                    scale=S8,
                )

            def emit_transpose(sidx, pt, g):
                emit_tp_mms(sidx, pt, g)
                emit_tp_copy()

            pend_C = []  # (htr, rzr, nwr, pt, sidx, want_transpose)

            def emit_C(htr_, rzr_, nwr_, pt_, sidx, want_tr):
                # C: combine  hn = nw + z*(ht-nw)  on full [128, 2048] rows
                d_ = wp.tile([128, N], bf16, tag="d")
                nc.gpsimd.tensor_sub(d_[:], htr_[:], nwr_[:])
                e_ = wp.tile([128, N], bf16, tag="e")
                nc.gpsimd.tensor_mul(e_[:], rzr_[:, 1, :], d_[:])
                nc.vector.tensor_add(hS[sidx][:, pt_, :], e_[:], nwr_[:])
                if want_tr:
                    pend_tr.append((sidx, pt_, 0))
                    pend_tr.append((sidx, pt_, 1))

            for s in range(T + P):
                cur, nxt = s % 2, (s + 1) % 2
                dec = s >= T
                last = s == T + P - 1
                hT8c = hT8[cur]
                hSp, hSn = hS[cur], hS[nxt]
                def emit_fc():
                    # fc head on previous state: out = fc . h + fc_b
                    for mc in range(NMC):
                        ms = slice(mc * 512, (mc + 1) * 512)
                        fcps = fcpool.tile([BC, 512], f32, tag="fc")
                        for pt_ in range(NPT):
                            nc.tensor.matmul(
                                fcps[:], fcblk[:, pt_, :], hSp[:, pt_, ms],
                                start=(pt_ == 0), stop=(pt_ == NPT - 1),
                            )
                        for g_ in range(3):
                            nc.scalar.activation(
                                ost[32 * g_:32 * g_ + 8, ms], fcps[:],
                                AF.Identity, bias=biases[0:8, 1:2],
                            )
                        nc.sync.dma_start(out_d[:, p_idx, ms], ost[0:8, ms])

                if not dec:
                    x_sb = xp.tile([128, N], bf16, tag="xsb")
                    for g in range(3):
                        eng = (nc.sync, nc.scalar, nc.sync)[g]
                        eng.dma_start(x_sb[32 * g:32 * g + 17, :],
                                      x_d[g, :, s, :])
                    inp_sb, inp_w = x_sb, wx17
                else:
                    p_idx = s - T
                    ost = osts[s % 2]
                    inp_sb, inp_w = ost, r9
                    if last:
                        emit_fc()
                        pend_tr.clear()  # transposes unneeded at the end
                        continue

                for pt in range(NPT):
                    psl = slice(pt * 128, (pt + 1) * 128)
                    # safety: any transposes this unit's diffusion needs that
                    # weren't emitted ahead go out now (MM + copy immediate)
                    while pend_tr and pend_tr[0][0] == cur and \
                            pend_tr[0][1] <= pt:
                        emit_transpose(*pend_tr.pop(0))
                    htr = rp.tile([128, N], bf16, tag="htr")
                    rzr = rp.tile([128, 2, N], bf16, tag="rzr")
                    nwr = rp.tile([128, N], bf16, tag="nwr")
                    # D/E: one long run of 32 DR matmuls per pt (keeps the
                    # weight-buffer pipeline + HAM warm), post-scale per
                    # 512-chunk double-buffered so D(c+1) overlaps E(c)
                    for mc in range(NMC):
                        ms = slice(mc * 512, (mc + 1) * 512)
                        dps = dpool.tile([128, 512], f32, tag="dps")
                        for kt2 in range(KT2):
                            nc.tensor.matmul(
                                dps[:], hT8c[:, kt2, :, psl],
                                mask_sb[:, kt2, :, ms],
                                start=(kt2 == 0), stop=(kt2 == KT2 - 1),
                                perf_mode=DR,
                            )
                        # ht = dps * dinv (diag trick: +h already inside)
                        nc.vector.tensor_mul(htr[:, ms], dps[:], dinv[:, ms])

                    if dec and pt == 0:
                        emit_fc()
                    # lagged transpose MMs fill PE while DVE runs E;
                    # their Scalar copy is emitted after the gate ACTs
                    if len(pend_tr) >= 3:
                        emit_tp_mms(*pend_tr.pop(0))

                    # G: gates per 512-chunk
                    for g in range(2):
                        t2p = wp.tile([128, 1024], bf16, tag="t2p")
                        for c in range(2):
                            ms = slice(g * 1024 + c * 512,
                                       g * 1024 + (c + 1) * 512)
                            rz = rzpool.tile([128, 2, 512], f32, tag="rz")
                            nups = npool.tile([128, 512], f32, tag="nu")
                            nwps = nwpool.tile([128, 512], f32, tag="nw")
                            kk = 17 if not dec else 9
                            # interleave U-gate and input matmuls (input MMs
                            # carry the gate bias via the ones row)
                            nc.tensor.matmul(rz[:, 0, :], ublk[:, 0, :],
                                             htr[:, ms], start=True, stop=False)
                            nc.tensor.matmul(
                                rz[:, 0, :], inp_w[0:kk, pt, :],
                                inp_sb[0:kk, ms], start=False, stop=True)
                            nc.tensor.matmul(rz[:, 1, :], ublk[:, 1, :],
                                             htr[:, ms], start=True, stop=False)
                            nc.tensor.matmul(
                                rz[:, 1, :], inp_w[32:32 + kk, pt, :],
                                inp_sb[32:32 + kk, ms], start=False, stop=True)
                            nc.tensor.matmul(nups[:], ublk[:, 2, :],
                                             htr[:, ms], start=True, stop=True)
                            nc.tensor.matmul(
                                nwps[:], inp_w[64:64 + kk, pt, :],
                                inp_sb[64:64 + kk, ms], start=True, stop=True)
                            # sigmoid over r|z pair, strided dst planes
                            nc.scalar.activation(rzr[:, :, ms], rz[:],
                                                 AF.Sigmoid)
                            # t1 = (nups + bn) * r ; t2 = t1 + nwps
                            t1 = wp.tile([128, 512], bf16, tag="t1")
                            nc.vector.scalar_tensor_tensor(
                                t1[:], nups[:], biases[:, 0:1], rzr[:, 0, ms],
                                op0=ALU.add, op1=ALU.mult,
                            )
                            nc.vector.tensor_add(
                                t2p[:, c * 512:(c + 1) * 512], t1[:], nwps[:])
                        nc.scalar.activation(
                            nwr[:, g * 1024:(g + 1) * 1024], t2p[:], AF.Tanh)
                        if pend_copy:
                            emit_tp_copy()
                        # second lagged transpose group after the first's
                        # copy has been emitted (tp PSUM slot is free)
                        if g == 0 and len(pend_tr) >= 3:
                            emit_tp_mms(*pend_tr.pop(0))

                    # combine of the PREVIOUS pt (lagged so this pt's DVE
                    # scale ops stay ahead in the queue)
                    if pend_C:
                        emit_C(*pend_C.pop(0))
                    pend_C.append((htr, rzr, nwr, pt, nxt, s < T + P - 2))
                for args in pend_C:
                    emit_C(*args)
                pend_C.clear()
            for it in pend_tr:
                emit_transpose(*it)
            pend_tr.clear()

    n_rm, n_cand = _dedup_ldweights(nc)
    nc.compile()
    nc._ldw_dedup_stats = (n_rm, n_cand)
    return nc


def _prep_consts(adj, Uw, Ub, Ww, Wb, fc_w, fc_bv):
    # Recover binary mask + column sums from adj = (I + 0.3*mask)/colsum.
    offd = adj.copy()
    np.fill_diagonal(offd, 0.0)
    vmax = offd.max(axis=0)
    diag = np.diagonal(adj).copy()
    cs = np.where(vmax > 0, 0.3 / np.maximum(vmax, 1e-30), 1.0 / diag)
    mask = (offd > 0).astype(np.float32)
    np.fill_diagonal(mask, (diag * cs > 1.15).astype(np.float32))
    dinv = (1.0 / cs).astype(np.float32)
    # maskI: exact fp8 pair — offd 1.125, diag += 3.75 (state scale 1/3.75)
    maskI = 1.125 * mask
    idx = np.arange(N)
    maskI[idx, idx] = maskI[idx, idx] + 3.75
    maskdr = np.ascontiguousarray(
        maskI.reshape(KT2, 2, 128, N).transpose(2, 0, 1, 3)
    ).astype(_F8)
    dinv_t = np.ascontiguousarray(
        np.broadcast_to(dinv[None, :], (128, N))
    ).astype(_BF16)

    ublk = np.zeros((128, 3, 128), np.float32)
    for g in range(3):
        for bl in range(2):
            sl = slice(bl * H, (bl + 1) * H)
            ublk[sl, g, sl] = Uw[g].T
    # encoder input lhsT with ones-row bias (x staged at partitions 32g+r)
    wx17 = np.zeros((128, NPT, 128), np.float32)
    r9 = np.zeros((128, NPT, 128), np.float32)
    for g in range(3):
        gb = (Ub[g] + Wb[g]) if g < 2 else Wb[2].copy()
        for pt in range(NPT):
            for bl in range(2):
                b = 2 * pt + bl
                csl = slice(bl * H, (bl + 1) * H)
                for f in range(F):
                    wx17[32 * g + 2 * b + f, pt, csl] = Ww[g][:, f]
                r9[32 * g + b, pt, csl] = Ww[g][:, 0]
            for bl in range(2):
                csl = slice(bl * H, (bl + 1) * H)
                wx17[32 * g + 16, pt, csl] = gb
                r9[32 * g + 8, pt, csl] = gb
    fcblk = np.zeros((128, NPT, BC), np.float32)
    for pt in range(NPT):
        for bl in range(2):
            fcblk[bl * H:(bl + 1) * H, pt, 2 * pt + bl] = fc_w
    biases = np.zeros((128, 2), np.float32)
    for bl in range(2):
        biases[bl * H:(bl + 1) * H, 0] = Ub[2]
    biases[:, 1] = fc_bv
    return dict(
        maskdr=maskdr, dinv=dinv_t,
        ublk=ublk.astype(_BF16), wx17=wx17.astype(_BF16),
        r9=r9.astype(_BF16), fcblk=fcblk.astype(_BF16),
        biases=biases, ident=np.eye(128, dtype=_BF16),
    )


def _prep_core_inputs(x, hidden0, consts):
    # x shard [BC, T, F*N] -> xfull [3, 17, T, N] with ones at row 16
    xr = x.reshape(BC, T, F, N).transpose(0, 2, 1, 3).reshape(BC * F, T, N)
    xfull = np.empty((3, 17, T, N), np.float32)
    xfull[:, :16] = xr[None, :, :, :]
    xfull[:, 16] = 1.0
    # hidden0 shard [BC, H, N] -> hS [128=(bl,h), NPT, N]
    h0r = np.ascontiguousarray(
        hidden0.reshape(NPT, 2, H, N).transpose(1, 2, 0, 3).reshape(128, NPT, N)
    ).astype(_BF16)
    # initial transposed fp8 state: hT8[ki, kt2, j, pt*128+blh] =
    #   S8 * h0r[blh, pt, (2kt2+j)*128+ki]
    h0f = np.asarray(h0r, np.float32) * S8           # [128, NPT, N]
    t = h0f.reshape(128, NPT, KT2, 2, 128)            # blh, pt, kt2, j, ki
    h0t8 = np.ascontiguousarray(
        t.transpose(4, 2, 3, 1, 0).reshape(128, KT2, 2, NPT * 128)
    ).astype(_F8)
    return dict(xfull=np.ascontiguousarray(xfull).astype(_BF16),
                h0=h0r, h0t8=h0t8, **consts)


def kernel(x, hidden0, adj, Ur_w, Ur_b, Wr_w, Wr_b, Uz_w, Uz_b, Wz_w, Wz_b,
           Un_w, Un_b, Wn_w, Wn_b, fc_w, fc_b, horizon):
    global _compiled
    from concourse.bass_utils import run_bass_kernel_spmd

    assert int(horizon) == P
    x = np.asarray(x, np.float32)
    hidden0 = np.asarray(hidden0, np.float32)
    adj = np.asarray(adj, np.float32)

    Uw = [np.asarray(w, np.float32) for w in (Ur_w, Uz_w, Un_w)]
    Ww = [np.asarray(w, np.float32) for w in (Wr_w, Wz_w, Wn_w)]
    Ub = [np.asarray(b, np.float32) for b in (Ur_b, Uz_b, Un_b)]
    Wb = [np.asarray(b, np.float32) for b in (Wr_b, Wz_b, Wn_b)]
    fc_w = np.asarray(fc_w, np.float32).reshape(H)
    fc_bv = float(np.asarray(fc_b, np.float32).reshape(()))

    consts = _prep_consts(adj, Uw, Ub, Ww, Wb, fc_w, fc_bv)

    if _compiled is None:
        _compiled = _build_bass()
    nc = _compiled

    in_maps = [
        _prep_core_inputs(x[c * BC:(c + 1) * BC],
                          hidden0[c * BC:(c + 1) * BC], consts)
        for c in range(NCORES)
    ]
    res = run_bass_kernel_spmd(nc, in_maps, core_ids=list(range(NCORES)))
    out = np.concatenate([res.results[c]["out"] for c in range(NCORES)], axis=0)
    return out.astype(np.float32)


# revision 30
# speedup vs baseline: 1.0756x; 1.0756x over previous
"""Trainium2 Bass kernel for a graph-GRU (GRNN) forecast model.

Math (per batch b, node m, hidden h; N=2048, H=64, F=2, T=12, P=6):
  ht[b,m,:] = sum_n adj[n,m] * h[b,:,n]           (graph diffusion + transpose)
  r = sig(ht@Ur^T + inp@Wr^T + br); z = sig(...); nw = tanh(r*(ht@Un^T+bn1) + inp@Wn^T + bn2)
  h' = (1-z)*nw + z*ht
Encoder: inp = x_t (T steps). Decoder: out = fc(h); inp = [out, 0] (P steps).

Data-parallel over batch: 8 cores x 8 batches. Per-core layouts:
  hT8 [n=2048(part, 8 DR pair-tiles), (b,h)=512(free)] fp8 = h/3.75
  hS  [(bl,h)=128(part), pt=4, m=2048]  bf16
with b = 2*pt + bl.

Diffusion: adj = (I + 0.3*mask)/colsum. The fp8 mask holds EXACT values
offd=1.125, diag+=3.75 and the state is scaled h/3.75, so
  dps = (h/3.75) @ maskI = (h + 0.3*mask@h)  exactly in coefficient
  ht  = dps * dinv[m]     one DVE mult (no separate +h add)
Per (pt, m-1024-group): 8 DR matmuls, one LDWEIGHTS per kt2 (the
duplicate LDW for the second 512-chunk is removed by post-schedule
surgery in _dedup_ldweights).

Gates: rz 2-bank PSUM pair -> ONE sigmoid ACT [128,1024] with strided
dst (r/z planes of rz_row). All gate biases are folded into the input
matmuls via a ones-row (x/out staged at partitions 32g..32g+16 for
3-row-group-concurrent input matmuls). nw tanh runs [128,1024] per
group. Combine d/e/hn run [128,1024] split DVE/GpSimd. Transposes
(PE+identity) lag 2 units; scalar copies tp->fp8 with scale 1/3.75.
"""

import numpy as np
import ml_dtypes

B, T, F, N, H, P = 64, 12, 2, 2048, 64, 6
NCORES = 8
BC = B // NCORES          # batches per core = 8
NPT = BC // 2             # batch-pair tiles = 4
KT2 = 8                   # DoubleRow pair tiles
NMC = 4                   # m chunks of 512
NU = 8                    # units per step: (pt, grp) with grp = m-1024 half

_BF16 = ml_dtypes.bfloat16
_F8 = ml_dtypes.float8_e4m3fn
S8 = 1.0 / 3.75           # fp8 state scale

_compiled = None


def _ldw_sig(inst):
    try:
        ap = inst.ins[0]
        return (str(ap.memref), int(ap.offset), str(ap.ap), str(ap.dtype),
                str(inst.perf_mode), str(inst.is_transpose))
    except Exception:
        return None


def _dedup_ldweights(nc):
    """Remove InstLdweights that reload the identical stationary operand.

    Scans each block's (program-ordered) instruction list; an LDW whose
    lowered weights-AP matches the previous PE weight load — with only
    MATMUL/EventSemaphore PE instructions in between — is redundant.
    Only removes wait-free, update-free instances (keeps all sync
    semantics intact). Returns (#removed, #candidates).
    """
    removed = cand = 0
    for fn in nc.m.functions:
        for blk in fn.blocks:
            il = blk.instructions
            keep = []
            changed = False
            prev_sig = None
            for inst in il:
                t = type(inst).__name__
                eng = str(getattr(inst, 'engine', ''))
                if 'PE' not in eng:
                    keep.append(inst)
                    continue
                if t == 'InstLdweights':
                    sig = _ldw_sig(inst)
                    if sig is not None and sig == prev_sig:
                        cand += 1
                        try:
                            clean = (not inst.has_wait()
                                     and not inst.has_update())
                        except Exception:
                            clean = False
                        if clean:
                            removed += 1
                            changed = True
                            continue
                    prev_sig = sig
                elif t in ('InstMatmult', 'InstEventSemaphore'):
                    pass
                else:
                    prev_sig = None
                keep.append(inst)
            if changed:
                il.clear()
                il.extend(keep)
    return removed, cand


def _build_bass():
    import concourse.bass as bass
    import concourse.mybir as mybir
    from concourse import bacc
    import concourse.tile as tile

    bf16 = mybir.dt.bfloat16
    f32 = mybir.dt.float32
    f8 = mybir.dt.float8e4
    AF = mybir.ActivationFunctionType
    ALU = mybir.AluOpType
    DR = mybir.MatmulPerfMode.DoubleRow

    nc = bacc.Bacc(None, target_bir_lowering=False)

    x_d = nc.dram_tensor("xfull", [3, 17, T, N], bf16, kind="ExternalInput")
    h0_d = nc.dram_tensor("h0", [128, NPT, N], bf16, kind="ExternalInput")
    h0t_d = nc.dram_tensor("h0t8", [128, KT2, 2, 512], f8, kind="ExternalInput")
    mask_d = nc.dram_tensor("maskdr", [128, KT2, 2, N], f8, kind="ExternalInput")
    dinv_d = nc.dram_tensor("dinv", [128, N], bf16, kind="ExternalInput")
    ublk_d = nc.dram_tensor("ublk", [128, 3, 128], bf16, kind="ExternalInput")
    wx_d = nc.dram_tensor("wx17", [128, NPT, 128], bf16, kind="ExternalInput")
    r9_d = nc.dram_tensor("r9", [128, NPT, 128], bf16, kind="ExternalInput")
    fc_d = nc.dram_tensor("fcblk", [128, NPT, BC], bf16, kind="ExternalInput")
    bias_d = nc.dram_tensor("biases", [128, 2], f32, kind="ExternalInput")
    id_d = nc.dram_tensor("ident", [128, 128], bf16, kind="ExternalInput")
    out_d = nc.dram_tensor("out", [BC, P, N], bf16, kind="ExternalOutput")

    with tile.TileContext(nc) as tc:
        with (
            tc.tile_pool(name="const", bufs=1) as cp,
            tc.tile_pool(name="state", bufs=1) as sp,
            tc.tile_pool(name="rowp", bufs=2) as rp,
            tc.tile_pool(name="work", bufs=3) as wp,
            tc.tile_pool(name="xp", bufs=2) as xp,
            tc.tile_pool(name="dps", bufs=1, space="PSUM") as dpool,
            tc.tile_pool(name="rzps", bufs=2, space="PSUM") as rzpool,
            tc.tile_pool(name="nups", bufs=1, space="PSUM") as npool,
            tc.tile_pool(name="nwps", bufs=1, space="PSUM") as nwpool,
            tc.tile_pool(name="tps", bufs=1, space="PSUM") as tpool,
        ):
            h0s = sp.tile([128, NPT, N], bf16, tag="hS0", name="hS0")
            nc.sync.dma_start(h0s[:], h0_d[:])
            hT80 = sp.tile([128, KT2, 2, 512], f8, tag="hT0", name="hT0")
            nc.scalar.dma_start(hT80[:], h0t_d[:])
            ident = cp.tile([128, 128], bf16)
            nc.scalar.dma_start(ident[:], id_d[:])
            mask_sb = cp.tile([128, KT2, 2, N], f8)
            for kt2 in range(KT2):
                eng = nc.sync if kt2 % 2 else nc.scalar
                eng.dma_start(mask_sb[:, kt2, :, :], mask_d[:, kt2, :, :])
            dinv = cp.tile([128, N], bf16)
            nc.scalar.dma_start(dinv[:], dinv_d[:])
            ublk = cp.tile([128, 3, 128], bf16)
            nc.scalar.dma_start(ublk[:], ublk_d[:])
            wx17 = cp.tile([128, NPT, 128], bf16)
            nc.scalar.dma_start(wx17[:], wx_d[:])
            r9 = cp.tile([128, NPT, 128], bf16)
            nc.scalar.dma_start(r9[:], r9_d[:])
            fcblk = cp.tile([128, NPT, BC], bf16)
            nc.scalar.dma_start(fcblk[:], fc_d[:])
            biases = cp.tile([128, 2], f32)
            nc.scalar.dma_start(biases[:], bias_d[:])

            hS = [h0s, sp.tile([128, NPT, N], bf16, tag="hS1", name="hS1")]
            hT8 = [hT80, sp.tile([128, KT2, 2, 512], f8, tag="hT1", name="hT1")]
            # decoder staged outputs: partitions {32g..32g+7} data, {32g+8} ones
            osts = [sp.tile([128, N], bf16, tag=f"ost{i}", name=f"ost{i}")
                    for i in range(2)]
            for o in osts:
                for g in range(3):
                    # ones row lives at 32g+8; rows 32g..32g+7 are data
                    # (rewritten by the fc ACT each decoder step) and rows
                    # 9..31 of each group are never read.
                    nc.vector.memset(o[32 * g:32 * (g + 1), :], 1.0)

            pend_tr = []    # (src_state_idx s%2 of hSn, pt, g)
            pend_copy = []  # (tpl_tile, sidx, pt, g)

            def emit_tp_mms(sidx, pt, g):
                hSn_ = hS[sidx]
                tpl = tpool.tile([128, 4, 2, 128], bf16, tag="tp")
                for j in range(8):
                    nc.tensor.transpose(
                        tpl[:, j // 2, j % 2, :],
                        hSn_[:, pt, g * 1024 + j * 128: g * 1024 + (j + 1) * 128],
                        ident[:],
                    )
                pend_copy.append((tpl, sidx, pt, g))

            def emit_tp_copy():
                tpl, sidx, pt, g = pend_copy.pop(0)
                psl = slice(pt * 128, (pt + 1) * 128)
                nc.scalar.activation(
                    hT8[sidx][:, 4 * g:4 * g + 4, :, psl], tpl[:], AF.Copy,
                    scale=S8,
                )

            def emit_transpose(sidx, pt, g):
                emit_tp_mms(sidx, pt, g)
                emit_tp_copy()

            pend_C = []  # (htr, rzr, nwr, pt, g, sidx, want_transpose)

            def emit_C(htr_, rzr_, nwr_, pt_, g_, sidx, want_tr):
                # C: combine  hn = nw + z*(ht-nw)
                gsl_ = slice(g_ * 1024, (g_ + 1) * 1024)
                d_ = wp.tile([128, 1024], bf16, tag="d")
                nc.gpsimd.tensor_sub(d_[:], htr_[:, gsl_], nwr_[:, gsl_])
                e_ = wp.tile([128, 1024], bf16, tag="e")
                nc.gpsimd.tensor_mul(e_[:], rzr_[:, 1, gsl_], d_[:])
                nc.vector.tensor_add(hS[sidx][:, pt_, gsl_], e_[:],
                                     nwr_[:, gsl_])
                if want_tr:
                    pend_tr.append((sidx, pt_, g_))

            for s in range(T + P):
                cur, nxt = s % 2, (s + 1) % 2
                dec = s >= T
                last = s == T + P - 1
                hT8c = hT8[cur]
                hSp, hSn = hS[cur], hS[nxt]
                def emit_fc():
                    # fc head on previous state: out = fc . h + fc_b
                    # (borrows the nwps PSUM slot: same tag + shape)
                    for mc in range(NMC):
                        ms = slice(mc * 512, (mc + 1) * 512)
                        fcps = nwpool.tile([128, 512], f32, tag="nw")
                        for pt_ in range(NPT):
                            nc.tensor.matmul(
                                fcps[0:BC, :], fcblk[:, pt_, :],
                                hSp[:, pt_, ms],
                                start=(pt_ == 0), stop=(pt_ == NPT - 1),
                            )
                        for g_ in range(3):
                            nc.scalar.activation(
                                ost[32 * g_:32 * g_ + 8, ms], fcps[0:BC, :],
                                AF.Identity, bias=biases[0:8, 1:2],
                            )
                        nc.sync.dma_start(out_d[:, p_idx, ms], ost[0:8, ms])

                if not dec:
                    x_sb = xp.tile([128, N], bf16, tag="xsb")
                    for g in range(3):
                        eng = (nc.sync, nc.scalar, nc.sync)[g]
                        eng.dma_start(x_sb[32 * g:32 * g + 17, :],
                                      x_d[g, :, s, :])
                    inp_sb, inp_w = x_sb, wx17
                else:
                    p_idx = s - T
                    ost = osts[s % 2]
                    inp_sb, inp_w = ost, r9
                    if last:
                        emit_fc()
                        pend_tr.clear()  # transposes unneeded at the end
                        continue

                for pt in range(NPT):
                    psl = slice(pt * 128, (pt + 1) * 128)
                    for g in range(2):
                        gsl = slice(g * 1024, (g + 1) * 1024)
                        if g == 0:
                            htr = rp.tile([128, N], bf16, tag="htr")
                            rzr = rp.tile([128, 2, N], bf16, tag="rzr")
                            nwr = rp.tile([128, N], bf16, tag="nwr")
                        # D/E: diffusion + post-scale per 512-chunk; the
                        # lagged transpose MMs fill PE while DVE runs E(c0)
                        # (dps bufs=1: D(c1) waits on E(c0))
                        mss = [slice(g * 1024 + c * 512,
                                     g * 1024 + (c + 1) * 512) for c in (0, 1)]
                        for c in range(2):
                            dps = dpool.tile([128, 512], f32, tag="dps")
                            for kt2 in range(KT2):
                                nc.tensor.matmul(
                                    dps[:], hT8c[:, kt2, :, psl],
                                    mask_sb[:, kt2, :, mss[c]],
                                    start=(kt2 == 0), stop=(kt2 == KT2 - 1),
                                    perf_mode=DR,
                                )
                            # ht = dps * dinv (diag trick: +h already inside)
                            nc.vector.tensor_mul(htr[:, mss[c]], dps[:],
                                                 dinv[:, mss[c]])
                            if c == 0 and len(pend_tr) >= 2:
                                emit_tp_mms(*pend_tr.pop(0))

                        if dec and pt == 0 and g == 0:
                            emit_fc()

                        # G: both chunks' input trios first (row groups
                        # overlap), then U-gate MMs + ACT chains per chunk
                        kk = 17 if not dec else 9
                        t2p = wp.tile([128, 1024], bf16, tag="t2p")
                        rzs = [rzpool.tile([128, 2, 512], f32, tag="rz",
                                           name=f"rz{i}") for i in range(2)]
                        nc.tensor.matmul(
                            rzs[0][:, 0, :], inp_w[0:kk, pt, :],
                            inp_sb[0:kk, mss[0]], start=True, stop=False)
                        nc.tensor.matmul(
                            rzs[0][:, 1, :], inp_w[32:32 + kk, pt, :],
                            inp_sb[32:32 + kk, mss[0]], start=True, stop=False)
                        nwps0 = nwpool.tile([128, 512], f32, tag="nw")
                        nc.tensor.matmul(
                            nwps0[:], inp_w[64:64 + kk, pt, :],
                            inp_sb[64:64 + kk, mss[0]], start=True, stop=True)
                        nc.tensor.matmul(
                            rzs[1][:, 0, :], inp_w[0:kk, pt, :],
                            inp_sb[0:kk, mss[1]], start=True, stop=False)
                        nc.tensor.matmul(
                            rzs[1][:, 1, :], inp_w[32:32 + kk, pt, :],
                            inp_sb[32:32 + kk, mss[1]], start=True, stop=False)
                        for c in range(2):
                            ms = mss[c]
                            rz = rzs[c]
                            nc.tensor.matmul(rz[:, 0, :], ublk[:, 0, :],
                                             htr[:, ms], start=False, stop=True)
                            nc.tensor.matmul(rz[:, 1, :], ublk[:, 1, :],
                                             htr[:, ms], start=False, stop=True)
                            nups = npool.tile([128, 512], f32, tag="nu")
                            nc.tensor.matmul(nups[:], ublk[:, 2, :],
                                             htr[:, ms], start=True, stop=True)
                            if c == 1:
                                nwps = nwpool.tile([128, 512], f32, tag="nw")
                                nc.tensor.matmul(
                                    nwps[:], inp_w[64:64 + kk, pt, :],
                                    inp_sb[64:64 + kk, ms],
                                    start=True, stop=True)
                            else:
                                nwps = nwps0
                            # sigmoid over r|z pair, strided dst planes
                            nc.scalar.activation(rzr[:, :, ms], rz[:],
                                                 AF.Sigmoid)
                            # t1 = (nups + bn) * r ; t2 = t1 + nwps
                            t1 = wp.tile([128, 512], bf16, tag="t1")
                            nc.vector.scalar_tensor_tensor(
                                t1[:], nups[:], biases[:, 0:1], rzr[:, 0, ms],
                                op0=ALU.add, op1=ALU.mult,
                            )
                            nc.vector.tensor_add(
                                t2p[:, c * 512:(c + 1) * 512], t1[:], nwps[:])
                        nc.scalar.activation(nwr[:, gsl], t2p[:], AF.Tanh)
                        if pend_copy:
                            emit_tp_copy()

                        # combine of the PREVIOUS unit (lagged so this
                        # unit's DVE scale op stays ahead in the queue)
                        if pend_C:
                            emit_C(*pend_C.pop(0))
                        pend_C.append((htr, rzr, nwr, pt, g, nxt,
                                       s < T + P - 2))
                for args in pend_C:
                    emit_C(*args)
                pend_C.clear()
            for it in pend_tr:
                emit_transpose(*it)
            pend_tr.clear()

    n_rm, n_cand = _dedup_ldweights(nc)
    nc.compile()
    nc._ldw_dedup_stats = (n_rm, n_cand)
    return nc


def _prep_consts(adj, Uw, Ub, Ww, Wb, fc_w, fc_bv):
    # Recover binary mask + column sums from adj = (I + 0.3*mask)/colsum.
    offd = adj.copy()
    np.fill_diagonal(offd, 0.0)
    vmax = offd.max(axis=0)
    diag = np.diagonal(adj).copy()
    cs = np.where(vmax > 0, 0.3 / np.maximum(vmax, 1e-30), 1.0 / diag)
    mask = (offd > 0).astype(np.float32)
    np.fill_diagonal(mask, (diag * cs > 1.15).astype(np.float32))
    dinv = (1.0 / cs).astype(np.float32)
    # maskI: exact fp8 pair — offd 1.125, diag += 3.75 (state scale 1/3.75)
    maskI = 1.125 * mask
    idx = np.arange(N)
    maskI[idx, idx] = maskI[idx, idx] + 3.75
    maskdr = np.ascontiguousarray(
        maskI.reshape(KT2, 2, 128, N).transpose(2, 0, 1, 3)
    ).astype(_F8)
    dinv_t = np.ascontiguousarray(
        np.broadcast_to(dinv[None, :], (128, N))
    ).astype(_BF16)

    ublk = np.zeros((128, 3, 128), np.float32)
    for g in range(3):
        for bl in range(2):
            sl = slice(bl * H, (bl + 1) * H)
            ublk[sl, g, sl] = Uw[g].T
    # encoder input lhsT with ones-row bias (x staged at partitions 32g+r)
    wx17 = np.zeros((128, NPT, 128), np.float32)
    r9 = np.zeros((128, NPT, 128), np.float32)
    for g in range(3):
        gb = (Ub[g] + Wb[g]) if g < 2 else Wb[2].copy()
        for pt in range(NPT):
            for bl in range(2):
                b = 2 * pt + bl
                csl = slice(bl * H, (bl + 1) * H)
                for f in range(F):
                    wx17[32 * g + 2 * b + f, pt, csl] = Ww[g][:, f]
                r9[32 * g + b, pt, csl] = Ww[g][:, 0]
            for bl in range(2):
                csl = slice(bl * H, (bl + 1) * H)
                wx17[32 * g + 16, pt, csl] = gb
                r9[32 * g + 8, pt, csl] = gb
    fcblk = np.zeros((128, NPT, BC), np.float32)
    for pt in range(NPT):
        for bl in range(2):
            fcblk[bl * H:(bl + 1) * H, pt, 2 * pt + bl] = fc_w
    biases = np.zeros((128, 2), np.float32)
    for bl in range(2):
        biases[bl * H:(bl + 1) * H, 0] = Ub[2]
    biases[:, 1] = fc_bv
    return dict(
        maskdr=maskdr, dinv=dinv_t,
        ublk=ublk.astype(_BF16), wx17=wx17.astype(_BF16),
        r9=r9.astype(_BF16), fcblk=fcblk.astype(_BF16),
        biases=biases, ident=np.eye(128, dtype=_BF16),
    )


def _prep_core_inputs(x, hidden0, consts):
    # x shard [BC, T, F*N] -> xfull [3, 17, T, N] with ones at row 16
    xr = x.reshape(BC, T, F, N).transpose(0, 2, 1, 3).reshape(BC * F, T, N)
    xfull = np.empty((3, 17, T, N), np.float32)
    xfull[:, :16] = xr[None, :, :, :]
    xfull[:, 16] = 1.0
    # hidden0 shard [BC, H, N] -> hS [128=(bl,h), NPT, N]
    h0r = np.ascontiguousarray(
        hidden0.reshape(NPT, 2, H, N).transpose(1, 2, 0, 3).reshape(128, NPT, N)
    ).astype(_BF16)
    # initial transposed fp8 state: hT8[ki, kt2, j, pt*128+blh] =
    #   S8 * h0r[blh, pt, (2kt2+j)*128+ki]
    h0f = np.asarray(h0r, np.float32) * S8           # [128, NPT, N]
    t = h0f.reshape(128, NPT, KT2, 2, 128)            # blh, pt, kt2, j, ki
    h0t8 = np.ascontiguousarray(
        t.transpose(4, 2, 3, 1, 0).reshape(128, KT2, 2, NPT * 128)
    ).astype(_F8)
    return dict(xfull=np.ascontiguousarray(xfull).astype(_BF16),
                h0=h0r, h0t8=h0t8, **consts)


def kernel(x, hidden0, adj, Ur_w, Ur_b, Wr_w, Wr_b, Uz_w, Uz_b, Wz_w, Wz_b,
           Un_w, Un_b, Wn_w, Wn_b, fc_w, fc_b, horizon):
    global _compiled
    from concourse.bass_utils import run_bass_kernel_spmd

    assert int(horizon) == P
    x = np.asarray(x, np.float32)
    hidden0 = np.asarray(hidden0, np.float32)
    adj = np.asarray(adj, np.float32)

    Uw = [np.asarray(w, np.float32) for w in (Ur_w, Uz_w, Un_w)]
    Ww = [np.asarray(w, np.float32) for w in (Wr_w, Wz_w, Wn_w)]
    Ub = [np.asarray(b, np.float32) for b in (Ur_b, Uz_b, Un_b)]
    Wb = [np.asarray(b, np.float32) for b in (Wr_b, Wz_b, Wn_b)]
    fc_w = np.asarray(fc_w, np.float32).reshape(H)
    fc_bv = float(np.asarray(fc_b, np.float32).reshape(()))

    consts = _prep_consts(adj, Uw, Ub, Ww, Wb, fc_w, fc_bv)

    if _compiled is None:
        _compiled = _build_bass()
    nc = _compiled

    in_maps = [
        _prep_core_inputs(x[c * BC:(c + 1) * BC],
                          hidden0[c * BC:(c + 1) * BC], consts)
        for c in range(NCORES)
    ]
    res = run_bass_kernel_spmd(nc, in_maps, core_ids=list(range(NCORES)))
    out = np.concatenate([res.results[c]["out"] for c in range(NCORES)], axis=0)
    return out.astype(np.float32)


# revision 31
# speedup vs baseline: 1.0906x; 1.0139x over previous
"""Trainium2 Bass kernel for a graph-GRU (GRNN) forecast model.

Math (per batch b, node m, hidden h; N=2048, H=64, F=2, T=12, P=6):
  ht[b,m,:] = sum_n adj[n,m] * h[b,:,n]           (graph diffusion + transpose)
  r = sig(ht@Ur^T + inp@Wr^T + br); z = sig(...); nw = tanh(r*(ht@Un^T+bn1) + inp@Wn^T + bn2)
  h' = (1-z)*nw + z*ht
Encoder: inp = x_t (T steps). Decoder: out = fc(h); inp = [out, 0] (P steps).

Data-parallel over batch: 8 cores x 8 batches. Per-core layouts:
  hT8 [n=2048(part, 8 DR pair-tiles), (b,h)=512(free)] fp8 = h/3.75
  hS  [(bl,h)=128(part), pt=4, m=2048]  bf16
with b = 2*pt + bl.

Diffusion: adj = (I + 0.3*mask)/colsum. The fp8 mask holds EXACT values
offd=1.125, diag+=3.75 and the state is scaled h/3.75, so
  dps = (h/3.75) @ maskI = (h + 0.3*mask@h)  exactly in coefficient
  ht  = dps * dinv[m]     one DVE mult (no separate +h add)
Per (pt, m-1024-group): 8 DR matmuls, one LDWEIGHTS per kt2 (the
duplicate LDW for the second 512-chunk is removed by post-schedule
surgery in _dedup_ldweights).

Gates: rz 2-bank PSUM pair -> ONE sigmoid ACT [128,1024] with strided
dst (r/z planes of rz_row). All gate biases are folded into the input
matmuls via a ones-row (x/out staged at partitions 32g..32g+16 for
3-row-group-concurrent input matmuls). nw tanh runs [128,1024] per
group. Combine d/e/hn run [128,1024] split DVE/GpSimd. Transposes
(PE+identity) lag 2 units; scalar copies tp->fp8 with scale 1/3.75.
"""

import numpy as np
import ml_dtypes

B, T, F, N, H, P = 64, 12, 2, 2048, 64, 6
NCORES = 8
BC = B // NCORES          # batches per core = 8
NPT = BC // 2             # batch-pair tiles = 4
KT2 = 8                   # DoubleRow pair tiles
NMC = 4                   # m chunks of 512
NU = 8                    # units per step: (pt, grp) with grp = m-1024 half

_BF16 = ml_dtypes.bfloat16
_F8 = ml_dtypes.float8_e4m3fn
S8 = 1.0 / 3.75           # fp8 state scale

_compiled = None


def _ldw_sig(inst):
    try:
        ap = inst.ins[0]
        return (str(ap.memref), int(ap.offset), str(ap.ap), str(ap.dtype),
                str(inst.perf_mode), str(inst.is_transpose))
    except Exception:
        return None


def _dedup_ldweights(nc):
    """Remove InstLdweights that reload the identical stationary operand.

    Scans each block's (program-ordered) instruction list; an LDW whose
    lowered weights-AP matches the previous PE weight load — with only
    MATMUL/EventSemaphore PE instructions in between — is redundant.
    Only removes wait-free, update-free instances (keeps all sync
    semantics intact). Returns (#removed, #candidates).
    """
    removed = cand = 0
    for fn in nc.m.functions:
        for blk in fn.blocks:
            il = blk.instructions
            keep = []
            changed = False
            prev_sig = None
            for inst in il:
                t = type(inst).__name__
                eng = str(getattr(inst, 'engine', ''))
                if 'PE' not in eng:
                    keep.append(inst)
                    continue
                if t == 'InstLdweights':
                    sig = _ldw_sig(inst)
                    if sig is not None and sig == prev_sig:
                        cand += 1
                        try:
                            clean = (not inst.has_wait()
                                     and not inst.has_update())
                        except Exception:
                            clean = False
                        if clean:
                            removed += 1
                            changed = True
                            continue
                    prev_sig = sig
                elif t in ('InstMatmult', 'InstEventSemaphore'):
                    pass
                else:
                    prev_sig = None
                keep.append(inst)
            if changed:
                il.clear()
                il.extend(keep)
    return removed, cand


def _build_bass():
    import concourse.bass as bass
    import concourse.mybir as mybir
    from concourse import bacc
    import concourse.tile as tile

    bf16 = mybir.dt.bfloat16
    f32 = mybir.dt.float32
    f8 = mybir.dt.float8e4
    AF = mybir.ActivationFunctionType
    ALU = mybir.AluOpType
    DR = mybir.MatmulPerfMode.DoubleRow

    nc = bacc.Bacc(None, target_bir_lowering=False)

    x_d = nc.dram_tensor("xfull", [3, 17, T, N], bf16, kind="ExternalInput")
    h0_d = nc.dram_tensor("h0", [128, NPT, N], bf16, kind="ExternalInput")
    h0t_d = nc.dram_tensor("h0t8", [128, KT2, 2, 512], f8, kind="ExternalInput")
    mask_d = nc.dram_tensor("maskdr", [128, KT2, 2, N], f8, kind="ExternalInput")
    dinv_d = nc.dram_tensor("dinv", [128, N], bf16, kind="ExternalInput")
    ublk_d = nc.dram_tensor("ublk", [128, 3, 128], bf16, kind="ExternalInput")
    wx_d = nc.dram_tensor("wx17", [128, NPT, 128], bf16, kind="ExternalInput")
    r9_d = nc.dram_tensor("r9", [128, NPT, 128], bf16, kind="ExternalInput")
    fc_d = nc.dram_tensor("fcblk", [128, NPT, BC], bf16, kind="ExternalInput")
    bias_d = nc.dram_tensor("biases", [128, 2], f32, kind="ExternalInput")
    id_d = nc.dram_tensor("ident", [128, 128], bf16, kind="ExternalInput")
    out_d = nc.dram_tensor("out", [BC, P, N], bf16, kind="ExternalOutput")

    with tile.TileContext(nc) as tc:
        with (
            tc.tile_pool(name="const", bufs=1) as cp,
            tc.tile_pool(name="state", bufs=1) as sp,
            tc.tile_pool(name="rowp", bufs=2) as rp,
            tc.tile_pool(name="work", bufs=3) as wp,
            tc.tile_pool(name="xp", bufs=2) as xp,
            tc.tile_pool(name="dps", bufs=2, space="PSUM") as dpool,
            tc.tile_pool(name="rzps", bufs=1, space="PSUM") as rzpool,
            tc.tile_pool(name="nups", bufs=1, space="PSUM") as npool,
            tc.tile_pool(name="nwps", bufs=1, space="PSUM") as nwpool,
            tc.tile_pool(name="tps", bufs=1, space="PSUM") as tpool,
            tc.tile_pool(name="fcps", bufs=1, space="PSUM") as fcpool,
        ):
            h0s = sp.tile([128, NPT, N], bf16, tag="hS0", name="hS0")
            nc.sync.dma_start(h0s[:], h0_d[:])
            hT80 = sp.tile([128, KT2, 2, 512], f8, tag="hT0", name="hT0")
            nc.scalar.dma_start(hT80[:], h0t_d[:])
            ident = cp.tile([128, 128], bf16)
            nc.scalar.dma_start(ident[:], id_d[:])
            mask_sb = cp.tile([128, KT2, 2, N], f8)
            for kt2 in range(KT2):
                eng = nc.sync if kt2 % 2 else nc.scalar
                eng.dma_start(mask_sb[:, kt2, :, :], mask_d[:, kt2, :, :])
            dinv = cp.tile([128, N], bf16)
            nc.scalar.dma_start(dinv[:], dinv_d[:])
            ublk = cp.tile([128, 3, 128], bf16)
            nc.scalar.dma_start(ublk[:], ublk_d[:])
            wx17 = cp.tile([128, NPT, 128], bf16)
            nc.scalar.dma_start(wx17[:], wx_d[:])
            r9 = cp.tile([128, NPT, 128], bf16)
            nc.scalar.dma_start(r9[:], r9_d[:])
            fcblk = cp.tile([128, NPT, BC], bf16)
            nc.scalar.dma_start(fcblk[:], fc_d[:])
            biases = cp.tile([128, 2], f32)
            nc.scalar.dma_start(biases[:], bias_d[:])

            hS = [h0s, sp.tile([128, NPT, N], bf16, tag="hS1", name="hS1")]
            hT8 = [hT80, sp.tile([128, KT2, 2, 512], f8, tag="hT1", name="hT1")]
            # decoder staged outputs: partitions {32g..32g+7} data, {32g+8} ones
            osts = [sp.tile([128, N], bf16, tag=f"ost{i}", name=f"ost{i}")
                    for i in range(2)]
            for o in osts:
                for g in range(3):
                    # ones row lives at 32g+8; rows 32g..32g+7 are data
                    # (rewritten by the fc ACT each decoder step) and rows
                    # 9..31 of each group are never read.
                    nc.vector.memset(o[32 * g:32 * (g + 1), :], 1.0)

            pend_tr = []    # (src_state_idx s%2 of hSn, pt, g)
            pend_copy = []  # (tpl_tile, sidx, pt, g)

            def emit_tp_mms(sidx, pt, g):
                hSn_ = hS[sidx]
                tpl = tpool.tile([128, 4, 2, 128], bf16, tag="tp")
                for j in range(8):
                    nc.tensor.transpose(
                        tpl[:, j // 2, j % 2, :],
                        hSn_[:, pt, g * 1024 + j * 128: g * 1024 + (j + 1) * 128],
                        ident[:],
                    )
                pend_copy.append((tpl, sidx, pt, g))

            def emit_tp_copy():
                tpl, sidx, pt, g = pend_copy.pop(0)
                psl = slice(pt * 128, (pt + 1) * 128)
                nc.scalar.activation(
                    hT8[sidx][:, 4 * g:4 * g + 4, :, psl], tpl[:], AF.Copy,
                    scale=S8,
                )

            def emit_transpose(sidx, pt, g):
                emit_tp_mms(sidx, pt, g)
                emit_tp_copy()

            pend_C = []  # (htr, rzr, nwr, pt, g, sidx, want_transpose)

            def emit_C(htr_, rzr_, nwr_, pt_, g_, sidx, want_tr):
                # C: combine  hn = nw + z*(ht-nw)
                gsl_ = slice(g_ * 1024, (g_ + 1) * 1024)
                d_ = wp.tile([128, 1024], bf16, tag="d")
                nc.gpsimd.tensor_sub(d_[:], htr_[:, gsl_], nwr_[:, gsl_])
                e_ = wp.tile([128, 1024], bf16, tag="e")
                nc.gpsimd.tensor_mul(e_[:], rzr_[:, 1, gsl_], d_[:])
                nc.vector.tensor_add(hS[sidx][:, pt_, gsl_], e_[:],
                                     nwr_[:, gsl_])
                if want_tr:
                    pend_tr.append((sidx, pt_, g_))

            for s in range(T + P):
                cur, nxt = s % 2, (s + 1) % 2
                dec = s >= T
                last = s == T + P - 1
                hT8c = hT8[cur]
                hSp, hSn = hS[cur], hS[nxt]
                def emit_fc():
                    # fc head on previous state: out = fc . h + fc_b
                    for mc in range(NMC):
                        ms = slice(mc * 512, (mc + 1) * 512)
                        fcps = fcpool.tile([BC, 512], f32, tag="fc")
                        for pt_ in range(NPT):
                            nc.tensor.matmul(
                                fcps[:], fcblk[:, pt_, :], hSp[:, pt_, ms],
                                start=(pt_ == 0), stop=(pt_ == NPT - 1),
                            )
                        for g_ in range(3):
                            nc.scalar.activation(
                                ost[32 * g_:32 * g_ + 8, ms], fcps[:],
                                AF.Identity, bias=biases[0:8, 1:2],
                            )
                        nc.sync.dma_start(out_d[:, p_idx, ms], ost[0:8, ms])

                if not dec:
                    x_sb = xp.tile([128, N], bf16, tag="xsb")
                    for g in range(3):
                        eng = (nc.sync, nc.scalar, nc.sync)[g]
                        eng.dma_start(x_sb[32 * g:32 * g + 17, :],
                                      x_d[g, :, s, :])
                    inp_sb, inp_w = x_sb, wx17
                else:
                    p_idx = s - T
                    ost = osts[s % 2]
                    inp_sb, inp_w = ost, r9
                    if last:
                        emit_fc()
                        pend_tr.clear()  # transposes unneeded at the end
                        continue

                for pt in range(NPT):
                    psl = slice(pt * 128, (pt + 1) * 128)
                    for g in range(2):
                        gsl = slice(g * 1024, (g + 1) * 1024)
                        if g == 0:
                            htr = rp.tile([128, N], bf16, tag="htr")
                            rzr = rp.tile([128, 2, N], bf16, tag="rzr")
                            nwr = rp.tile([128, N], bf16, tag="nwr")
                        # D/E: diffusion + post-scale per 512-chunk,
                        # double-buffered so D(c+1) overlaps E(c)
                        for c in range(2):
                            ms = slice(g * 1024 + c * 512,
                                       g * 1024 + (c + 1) * 512)
                            dps = dpool.tile([128, 512], f32, tag="dps")
                            for kt2 in range(KT2):
                                nc.tensor.matmul(
                                    dps[:], hT8c[:, kt2, :, psl],
                                    mask_sb[:, kt2, :, ms],
                                    start=(kt2 == 0), stop=(kt2 == KT2 - 1),
                                    perf_mode=DR,
                                )
                            # ht = dps * dinv (diag trick: +h already inside)
                            nc.vector.tensor_mul(htr[:, ms], dps[:],
                                                 dinv[:, ms])

                        if dec and pt == 0 and g == 0:
                            emit_fc()
                        # lagged transpose MMs fill PE while DVE runs E;
                        # their Scalar copy is emitted after the gate ACTs
                        if len(pend_tr) >= 2:
                            emit_tp_mms(*pend_tr.pop(0))

                        # G: gates for the two 512-chunks of this unit
                        t2p = wp.tile([128, 1024], bf16, tag="t2p")
                        for c in range(2):
                            ms = slice(g * 1024 + c * 512,
                                       g * 1024 + (c + 1) * 512)
                            rz = rzpool.tile([128, 2, 512], f32, tag="rz")
                            nups = npool.tile([128, 512], f32, tag="nu")
                            nwps = nwpool.tile([128, 512], f32, tag="nw")
                            kk = 17 if not dec else 9
                            # interleave U-gate and input matmuls (input MMs
                            # carry the gate bias via the ones row)
                            nc.tensor.matmul(rz[:, 0, :], ublk[:, 0, :],
                                             htr[:, ms], start=True, stop=False)
                            nc.tensor.matmul(
                                rz[:, 0, :], inp_w[0:kk, pt, :],
                                inp_sb[0:kk, ms], start=False, stop=True)
                            nc.tensor.matmul(rz[:, 1, :], ublk[:, 1, :],
                                             htr[:, ms], start=True, stop=False)
                            nc.tensor.matmul(
                                rz[:, 1, :], inp_w[32:32 + kk, pt, :],
                                inp_sb[32:32 + kk, ms], start=False, stop=True)
                            nc.tensor.matmul(nups[:], ublk[:, 2, :],
                                             htr[:, ms], start=True, stop=True)
                            nc.tensor.matmul(
                                nwps[:], inp_w[64:64 + kk, pt, :],
                                inp_sb[64:64 + kk, ms], start=True, stop=True)
                            # sigmoid over r|z pair, strided dst planes
                            nc.scalar.activation(rzr[:, :, ms], rz[:],
                                                 AF.Sigmoid)
                            # t1 = (nups + bn) * r ; t2 = t1 + nwps
                            t1 = wp.tile([128, 512], bf16, tag="t1")
                            nc.vector.scalar_tensor_tensor(
                                t1[:], nups[:], biases[:, 0:1], rzr[:, 0, ms],
                                op0=ALU.add, op1=ALU.mult,
                            )
                            nc.vector.tensor_add(
                                t2p[:, c * 512:(c + 1) * 512], t1[:], nwps[:])
                        nc.scalar.activation(nwr[:, gsl], t2p[:], AF.Tanh)
                        if pend_copy:
                            emit_tp_copy()

                        # combine of the PREVIOUS unit (lagged so this
                        # unit's DVE scale op stays ahead in the queue)
                        if pend_C:
                            emit_C(*pend_C.pop(0))
                        pend_C.append((htr, rzr, nwr, pt, g, nxt,
                                       s < T + P - 2))
                for args in pend_C:
                    emit_C(*args)
                pend_C.clear()
            for it in pend_tr:
                emit_transpose(*it)
            pend_tr.clear()

    n_rm, n_cand = _dedup_ldweights(nc)
    nc.compile()
    nc._ldw_dedup_stats = (n_rm, n_cand)
    return nc


def _prep_consts(adj, Uw, Ub, Ww, Wb, fc_w, fc_bv):
    # Recover binary mask + column sums from adj = (I + 0.3*mask)/colsum.
    offd = adj.copy()
    np.fill_diagonal(offd, 0.0)
    vmax = offd.max(axis=0)
    diag = np.diagonal(adj).copy()
    cs = np.where(vmax > 0, 0.3 / np.maximum(vmax, 1e-30), 1.0 / diag)
    mask = (offd > 0).astype(np.float32)
    np.fill_diagonal(mask, (diag * cs > 1.15).astype(np.float32))
    dinv = (1.0 / cs).astype(np.float32)
    # maskI: exact fp8 pair — offd 1.125, diag += 3.75 (state scale 1/3.75)
    maskI = 1.125 * mask
    idx = np.arange(N)
    maskI[idx, idx] = maskI[idx, idx] + 3.75
    maskdr = np.ascontiguousarray(
        maskI.reshape(KT2, 2, 128, N).transpose(2, 0, 1, 3)
    ).astype(_F8)
    dinv_t = np.ascontiguousarray(
        np.broadcast_to(dinv[None, :], (128, N))
    ).astype(_BF16)

    ublk = np.zeros((128, 3, 128), np.float32)
    for g in range(3):
        for bl in range(2):
            sl = slice(bl * H, (bl + 1) * H)
            ublk[sl, g, sl] = Uw[g].T
    # encoder input lhsT with ones-row bias (x staged at partitions 32g+r)
    wx17 = np.zeros((128, NPT, 128), np.float32)
    r9 = np.zeros((128, NPT, 128), np.float32)
    for g in range(3):
        gb = (Ub[g] + Wb[g]) if g < 2 else Wb[2].copy()
        for pt in range(NPT):
            for bl in range(2):
                b = 2 * pt + bl
                csl = slice(bl * H, (bl + 1) * H)
                for f in range(F):
                    wx17[32 * g + 2 * b + f, pt, csl] = Ww[g][:, f]
                r9[32 * g + b, pt, csl] = Ww[g][:, 0]
            for bl in range(2):
                csl = slice(bl * H, (bl + 1) * H)
                wx17[32 * g + 16, pt, csl] = gb
                r9[32 * g + 8, pt, csl] = gb
    fcblk = np.zeros((128, NPT, BC), np.float32)
    for pt in range(NPT):
        for bl in range(2):
            fcblk[bl * H:(bl + 1) * H, pt, 2 * pt + bl] = fc_w
    biases = np.zeros((128, 2), np.float32)
    for bl in range(2):
        biases[bl * H:(bl + 1) * H, 0] = Ub[2]
    biases[:, 1] = fc_bv
    return dict(
        maskdr=maskdr, dinv=dinv_t,
        ublk=ublk.astype(_BF16), wx17=wx17.astype(_BF16),
        r9=r9.astype(_BF16), fcblk=fcblk.astype(_BF16),
        biases=biases, ident=np.eye(128, dtype=_BF16),
    )


def _prep_core_inputs(x, hidden0, consts):
    # x shard [BC, T, F*N] -> xfull [3, 17, T, N] with ones at row 16
    xr = x.reshape(BC, T, F, N).transpose(0, 2, 1, 3).reshape(BC * F, T, N)
    xfull = np.empty((3, 17, T, N), np.float32)
    xfull[:, :16] = xr[None, :, :, :]
    xfull[:, 16] = 1.0
    # hidden0 shard [BC, H, N] -> hS [128=(bl,h), NPT, N]
    h0r = np.ascontiguousarray(
        hidden0.reshape(NPT, 2, H, N).transpose(1, 2, 0, 3).reshape(128, NPT, N)
    ).astype(_BF16)
    # initial transposed fp8 state: hT8[ki, kt2, j, pt*128+blh] =
    #   S8 * h0r[blh, pt, (2kt2+j)*128+ki]
    h0f = np.asarray(h0r, np.float32) * S8           # [128, NPT, N]
    t = h0f.reshape(128, NPT, KT2, 2, 128)            # blh, pt, kt2, j, ki
    h0t8 = np.ascontiguousarray(
        t.transpose(4, 2, 3, 1, 0).reshape(128, KT2, 2, NPT * 128)
    ).astype(_F8)
    return dict(xfull=np.ascontiguousarray(xfull).astype(_BF16),
                h0=h0r, h0t8=h0t8, **consts)


def kernel(x, hidden0, adj, Ur_w, Ur_b, Wr_w, Wr_b, Uz_w, Uz_b, Wz_w, Wz_b,
           Un_w, Un_b, Wn_w, Wn_b, fc_w, fc_b, horizon):
    global _compiled
    from concourse.bass_utils import run_bass_kernel_spmd

    assert int(horizon) == P
    x = np.asarray(x, np.float32)
    hidden0 = np.asarray(hidden0, np.float32)
    adj = np.asarray(adj, np.float32)

    Uw = [np.asarray(w, np.float32) for w in (Ur_w, Uz_w, Un_w)]
    Ww = [np.asarray(w, np.float32) for w in (Wr_w, Wz_w, Wn_w)]
    Ub = [np.asarray(b, np.float32) for b in (Ur_b, Uz_b, Un_b)]
    Wb = [np.asarray(b, np.float32) for b in (Wr_b, Wz_b, Wn_b)]
    fc_w = np.asarray(fc_w, np.float32).reshape(H)
    fc_bv = float(np.asarray(fc_b, np.float32).reshape(()))

    consts = _prep_consts(adj, Uw, Ub, Ww, Wb, fc_w, fc_bv)

    if _compiled is None:
        _compiled = _build_bass()
    nc = _compiled

    in_maps = [
        _prep_core_inputs(x[c * BC:(c + 1) * BC],
                          hidden0[c * BC:(c + 1) * BC], consts)
        for c in range(NCORES)
    ]
    res = run_bass_kernel_spmd(nc, in_maps, core_ids=list(range(NCORES)))
    out = np.concatenate([res.results[c]["out"] for c in range(NCORES)], axis=0)
    return out.astype(np.float32)
